# revision 53
# baseline (speedup 1.0000x reference)
"""Inner-policy-sharded Trainium2 kernel for DecoupledDynamicsModel (MoE).

Model: B=8192 rows; each row selects one of P=8 outer policies via
policy_indices; the selected policy runs 8 inner MLPs (72 -> 512 -> 512 -> 64)
on (latent chunk, action) and the 8 inner outputs concatenate to 512 dims.

Sharding: by INNER policy. Core i computes inner MLP i for every row, using
the row's outer-policy weight set W*[outer, i]. Rows are sorted by outer
policy on the host so tokens form 8 contiguous groups; within a group the
weights are stationary. Perfect load balance (every core runs exactly B
tokens), no capacity padding.

Design (93.2us -> 84.9us vs the fp32r baseline):
- bf16 operands everywhere (fp32 PSUM accumulate). Same 1 cycle/row tensor
  rate as fp32r, but half the DMA bytes, so the weight stream stays ahead
  of compute. End-to-end rel err ~4e-3, well inside the 2e-2 gate.
- Layer 3 swaps stationary/moving operands: h2 128-token slices are the
  stationary operand and W3 [128, 64] chunks stream. Moving rows per
  512-token tile drop from 4x512 to 16x64 (2 instead of 4 cycles/token,
  22 total vs 24), and the output lands token-major in PSUM. Each subtile
  is one k-contiguous PSUM accumulation group: start=True clears
  has_written bank-wide, so groups sharing a bank must not interleave.
- b3 is added on the host during the gather (it varies along the free dim
  in the token-major layout, which the per-partition bias port can't do).
- Warmup matmuls on a memset scratch tile ramp the PE p-state during the
  unavoidable ~3.5us DMA lead-in (barrier + DGE fixed costs + 900ns DMA
  semaphore), so real matmuls start at full clock. A dummy activation at
  t=0 pulls the ~1.3us ACT function-table load off the first eviction's
  critical path.
- W2 is packed m-major and group 0 is split into need-ordered DMA chunks
  so tile 0's L2 m-chunks chase the arrivals without stalling; groups 1-7
  load as single bulk DMAs, x spans interleaved on the SWDGE queue.
- Software-pipelined emission: L2(t) m-chunks interleave with L1(t+1)
  m-chunks (the ACT h1 evictions get a full tile of slack), L3(t-1)
  trails, and y DMAs alternate between the HWDGE and SWDGE queues. The
  last tile's h2 evictions all go to DVE in m-order so its L3 k-chunks
  chase them with minimal stall; the final tile is kept small (128 rows)
  to shorten the eviction -> y-copy -> DMA tail.
"""

import sys

sys.path.insert(0, "/opt/trn_rl_repo")

import numpy as np
import ml_dtypes

import concourse.bass as bass
from concourse import bacc
import concourse.mybir as mybir
import concourse.tile as tile
from concourse.bass_utils import run_bass_kernel_spmd

P = 8          # outer policies == n_cores == inner MLPs per policy
Z = 64         # per-policy latent dim
D = P * Z      # 512
A = 8          # action dim
IN = Z + A     # 72, MLP input dim
H = 512        # hidden dim
NCORES = 8

# column layout of the [128, WBC] packed per-outer bf16 weight tile
W1_OFF = 0            # [72, 512] (rows 72.. zero)
W2_OFF = 512          # m-major: m*512 + k*128, 4x4 chunks of [128, 128]
W3_OFF = 2560         # 4 k-chunks of [128, 64]
WBC = 2816

F32 = mybir.dt.float32
BF16 = mybir.dt.bfloat16
RELU = mybir.ActivationFunctionType.Relu
BF = ml_dtypes.bfloat16

NWARM = 23            # warmup matmuls covering the DMA lead-in
WARM_N = 128

TRACE = False
REPEAT = 1
LAST_RESULT = None


def _group_tiles(counts):
    """Token tiles for the sorted stream: each tile stays inside one outer-
    policy group; tiles <=512. The very last tile is kept small so the tail
    (final matmul -> y DMA) drains quickly."""
    tiles = []
    for g, n in enumerate(counts):
        off = sum(counts[:g])
        r = n
        last_group = g == len(counts) - 1
        while r > 0:
            if last_group:
                if r > 640:
                    t = 512
                elif r > 128:
                    t = r - 128
                else:
                    t = r
            elif r > 1024:
                t = 512
            elif r > 512:
                t = -(-r // 2 // 4) * 4
            else:
                t = r
            tiles.append((g, off, t))
            off += t
            r -= t
    # attach subtile base offsets for the y output layout
    out = []
    s0 = 0
    for (g, t0, tw) in tiles:
        ns = -(-tw // 128)
        out.append((g, t0, tw, s0, ns))
        s0 += ns
    return out, s0


def _build_program(counts, B, repeat=1):
    tiles, ntot = _group_tiles(counts)
    nc = bacc.Bacc()

    xTd = nc.declare_dram_parameter("xT", [IN, B], BF16, isOutput=False)
    wbd = nc.declare_dram_parameter("wb", [P, 128, WBC], BF16, isOutput=False)
    bsd = nc.declare_dram_parameter("bs", [128, P * 8], F32, isOutput=False)
    yd = nc.declare_dram_parameter("y", [128, ntot, Z], BF16, isOutput=True)

    with tile.TileContext(nc) as tc:
        with (
            tc.tile_pool(name="warm", bufs=1) as warmpool,
            tc.tile_pool(name="weights", bufs=8) as wpool,
            tc.tile_pool(name="xs", bufs=1) as xpool,
            tc.tile_pool(name="bias", bufs=1) as bpool,
            tc.tile_pool(name="hs", bufs=6) as hpool,
            tc.tile_pool(name="ys", bufs=3) as ypool,
            tc.tile_pool(name="ps1", bufs=3, space="PSUM") as pspool1,
            tc.tile_pool(name="ps2", bufs=3, space="PSUM") as pspool2,
            tc.tile_pool(name="ps3", bufs=2, space="PSUM") as pspool3,
        ):
            for _rep in range(repeat):
                # -- warmup: ramp the PE p-state while the first DMAs fly.
                # Reads an uninitialized SBUF tile; results land in scratch
                # PSUM that is cleared (start=True) before any real use.
                wt = warmpool.tile([128, 512], BF16, tag="warm")
                nc.vector.memset(wt[:, 0:WARM_N], 0.0)
                # dummy activation: pulls the ACT function-table load off the
                # first real eviction's critical path (it costs ~1.3us)
                sc = warmpool.tile([128, 1], F32, tag="sc")
                nc.scalar.activation(sc[:, 0:1], wt[:, 0:1], RELU)
                for _w in range(NWARM):
                    wps = pspool1.tile([128, 512], F32, tag="ps1")
                    nc.tensor.matmul(
                        wps[:, :WARM_N], wt[0:128, 0:128], wt[:, :WARM_N],
                        start=True, stop=True,
                    ).annotate(f"warm{_w}")

                xt = xpool.tile([IN, B], BF16, tag="x")
                bt = bpool.tile([128, P * 8], F32, tag="bias")
                wbs = []
                for _g in range(P):
                    wb_t = wpool.tile([128, WBC], BF16, tag="wb")
                    wbs.append(wb_t)

                # -- DMA emission in need order (transfers drain through one
                # serial pipe). x rides SWDGE (gpsimd) in parallel with the
                # HWDGE weight pipe's generation stages.
                x_cuts = sorted(set(min(c, B) for c in
                                    [0, 512, 1024, 2048, 3072, 4096, 5120,
                                     6144, 7168, B]))
                xsp = [c for c in zip(x_cuts[:-1], x_cuts[1:]) if c[1] > c[0]]
                for c0, c1 in xsp[:2]:
                    nc.gpsimd.dma_start(xt[:, c0:c1], xTd[:, c0:c1])
                # group 0 split so tile 0's L2 m-chunks chase the arrivals:
                # W1+W2m0 | bias | W2m1 | W2m2 | W2m3 | W3
                nc.sync.dma_start(wbs[0][:, 0:512], wbd[0, :, 0:512])
                nc.sync.dma_start(bt[:, 0: P * 8], bsd[:, 0: P * 8])
                nc.sync.dma_start(wbs[0][:, 512:1024], wbd[0, :, 512:1024])
                for m in range(1, 4):
                    c0 = W2_OFF + m * 512
                    nc.sync.dma_start(wbs[0][:, c0:c0 + 512],
                                      wbd[0, :, c0:c0 + 512])
                nc.sync.dma_start(wbs[0][:, W3_OFF:WBC], wbd[0, :, W3_OFF:WBC])
                # groups 1..7: one bulk DMA each, x spans interleaved
                for g in range(1, P):
                    nc.sync.dma_start(wbs[g][:, 0:WBC], wbd[g, :, 0:WBC])
                    xi = 1 + g
                    if xi < len(xsp):
                        c0, c1 = xsp[xi]
                        nc.gpsimd.dma_start(xt[:, c0:c1], xTd[:, c0:c1])

                # -- compute, software-pipelined: L2(t), L3(t-1), L1(t+1)
                h1s, h2s, ps3s = {}, {}, {}

                def emit_l1_m(ti, m):
                    g, t0, tw, _, _ = tiles[ti]
                    wb = wbs[g]
                    if m == 0:
                        h1_t = hpool.tile([128, 4, 512], BF16, tag="h1")
                        h1s[ti] = h1_t
                    h1 = h1s[ti]
                    if ti == 0 and m == 3:
                        # ps1 holds only 3 banks; tile 0 has no preceding L2
                        # work to hide the ev1-m0 WAR wait, so borrow the
                        # still-idle ps2 pool for its fourth m-chunk
                        ps = pspool2.tile([128, 512], F32, tag="ps2")
                    else:
                        ps = pspool1.tile([128, 512], F32, tag="ps1")
                    nc.tensor.matmul(
                        ps[:, :tw],
                        wb[0:IN, W1_OFF + m * 128: W1_OFF + (m + 1) * 128],
                        xt[:, t0: t0 + tw],
                        start=True, stop=True,
                    ).annotate(f"L1 t{ti} m{m}")
                    nc.scalar.activation(
                        h1[:, m, :tw], ps[:, :tw], RELU,
                        bias=bt[:, g * 8 + m: g * 8 + m + 1],
                    ).annotate(f"ev1 t{ti} m{m}")

                def emit_l2_m(ti, m):
                    g, t0, tw, _, _ = tiles[ti]
                    wb = wbs[g]
                    h1 = h1s[ti]
                    if m == 0:
                        h2_t = hpool.tile([128, 4, 512], BF16, tag="h2")
                        h2s[ti] = h2_t
                    h2 = h2s[ti]
                    ps = pspool2.tile([128, 512], F32, tag="ps2")
                    for k in range(4):
                        c0 = W2_OFF + m * 512 + k * 128
                        nc.tensor.matmul(
                            ps[:, :tw], wb[:, c0: c0 + 128],
                            h1[:, k, :tw],
                            start=(k == 0), stop=(k == 3),
                        ).annotate(f"L2 t{ti} m{m} k{k}")
                    # near the tail, split the h2 evictions across ACT+DVE:
                    # there is no following tile whose PE work could hide a
                    # serial single-engine drain
                    if ti == len(tiles) - 2 and m % 2 == 1:
                        nc.scalar.activation(
                            h2[:, m, :tw], ps[:, :tw], RELU,
                            bias=bt[:, g * 8 + 4 + m: g * 8 + 4 + m + 1],
                        ).annotate(f"ev2 t{ti} m{m}")
                    else:
                        nc.vector.tensor_scalar(
                            h2[:, m, :tw], ps[:, :tw],
                            bt[:, g * 8 + 4 + m: g * 8 + 4 + m + 1],
                            0.0,
                            mybir.AluOpType.add,
                            mybir.AluOpType.max,
                        ).annotate(f"ev2 t{ti} m{m}")

                def emit_l3_sub(ti, sub):
                    # one PSUM accumulation group per subtile, k-contiguous:
                    # start=True clears has_written bank-wide, so groups in
                    # the same bank must not interleave
                    g, t0, tw, s0, ns = tiles[ti]
                    if sub >= ns:
                        return
                    wb = wbs[g]
                    if sub == 0:
                        ps3_t = pspool3.tile([128, 4, Z], F32, tag="ps3")
                        ps3s[ti] = ps3_t
                    ps = ps3s[ti]
                    h2 = h2s[ti]
                    c0 = sub * 128
                    mm = min(128, tw - c0)
                    for k in range(4):
                        nc.tensor.matmul(
                            ps[0:mm, sub, :],
                            h2[:, k, c0: c0 + mm],
                            wb[:, W3_OFF + k * Z: W3_OFF + (k + 1) * Z],
                            start=(k == 0), stop=(k == 3),
                        ).annotate(f"L3 t{ti} s{sub} k{k}")

                def emit_y(ti):
                    g, t0, tw, s0, ns = tiles[ti]
                    ps = ps3s.pop(ti)
                    h2s.pop(ti)
                    yt = ypool.tile([128, 4, Z], BF16, tag="y")
                    if ti == len(tiles) - 1:
                        # tail: evict + DMA in two independent half-Z halves
                        # on both engines/queues so the ~2.6us fixed DMA
                        # latencies overlap instead of chaining
                        h = Z // 2
                        nc.vector.tensor_scalar(
                            yt[:, 0:ns, 0:h], ps[:, 0:ns, 0:h], 0.0, None,
                            mybir.AluOpType.add,
                        ).annotate(f"evy t{ti} a")
                        nc.scalar.activation(
                            yt[:, 0:ns, h:Z], ps[:, 0:ns, h:Z],
                            mybir.ActivationFunctionType.Copy,
                        ).annotate(f"evy t{ti} b")
                        nc.gpsimd.dma_start(
                            yd[:, s0: s0 + ns, 0:h], yt[:, 0:ns, 0:h]
                        ).annotate(f"dmay t{ti} a")
                        nc.sync.dma_start(
                            yd[:, s0: s0 + ns, h:Z], yt[:, 0:ns, h:Z]
                        ).annotate(f"dmay t{ti} b")
                    else:
                        nc.scalar.activation(
                            yt[:, 0:ns, :], ps[:, 0:ns, :],
                            mybir.ActivationFunctionType.Copy,
                        ).annotate(f"evy t{ti}")
                        q = nc.gpsimd if ti % 2 == 0 else nc.sync
                        q.dma_start(
                            yd[:, s0: s0 + ns, :], yt[:, 0:ns, :]
                        ).annotate(f"dmay t{ti}")

                # PE order per tile t: L2(t) m-chunks interleaved with
                # L1(t+1) m-chunks (gives the ACT h1 evictions a full tile
                # of slack), then L3(t-1). The last tile interleaves its own
                # L3 k-chunks behind the L2 m-chunks that feed them.
                nt = len(tiles)
                for m in range(4):
                    emit_l1_m(0, m)
                for ti in range(nt - 1):
                    for m in range(4):
                        emit_l2_m(ti, m)
                        emit_l1_m(ti + 1, m)
                    if ti > 0:
                        for sub in range(4):
                            emit_l3_sub(ti - 1, sub)
                        emit_y(ti - 1)
                # last tile: interleave L3(T-1) subtiles behind L2(T)
                T = nt - 1
                for m in range(4):
                    emit_l2_m(T, m)
                    if T > 0:
                        emit_l3_sub(T - 1, m)
                if T > 0:
                    emit_y(T - 1)
                for sub in range(4):
                    emit_l3_sub(T, sub)
                emit_y(T)

    nc.finalize()
    return nc, tiles, ntot


def _pack_inputs(latents, actions, order, counts, pcounts, Bp,
                 W1, b1, W2, b2, W3, b3):
    """Per-core inputs. Core i: xT = [latent chunk i; action] for all rows in
    sorted order (groups padded to pcounts); wb[g] = weights of (outer g,
    inner i), W2 packed m-major; bs = b1|b2 activation biases in fp32."""
    lat_s = latents[order]                       # [B, 512]
    act_s = actions[order]                       # [B, 8]
    spans = []                                   # (padded off, raw off, n)
    po = ro = 0
    for n, pn in zip(counts, pcounts):
        spans.append((po, ro, n))
        po += pn
        ro += n
    in_maps = []
    for i in range(NCORES):
        xT = np.zeros((IN, Bp), dtype=BF)
        for po, ro, n in spans:
            xT[:Z, po: po + n] = lat_s[ro: ro + n, i * Z: (i + 1) * Z].T.astype(BF)
            xT[Z:, po: po + n] = act_s[ro: ro + n].T.astype(BF)

        wb = np.zeros((P, 128, WBC), dtype=BF)
        wb[:, :IN, W1_OFF: W1_OFF + 512] = W1[:, i].astype(BF)   # [P, 72, 512]
        # W2[g, i]: [512(k), 512(m)] -> chunk (m, k) at col m*512 + k*128
        w2 = W2[:, i].reshape(P, 4, 128, 4, 128)          # [P, k4, 128, m4, 128]
        wb[:, :, W2_OFF: W2_OFF + 2048] = (
            w2.transpose(0, 2, 3, 1, 4).reshape(P, 128, 2048).astype(BF)
        )
        wb[:, :, W3_OFF: W3_OFF + 256] = (
            W3[:, i].reshape(P, 4, 128, Z).transpose(0, 2, 1, 3)
            .reshape(P, 128, 256).astype(BF)
        )
        bs = np.zeros((128, P * 8), dtype=np.float32)
        for g in range(P):
            bs[:, g * 8: g * 8 + 4] = b1[g, i].reshape(4, 128).T
            bs[:, g * 8 + 4: g * 8 + 8] = b2[g, i].reshape(4, 128).T

        in_maps.append({"xT": xT, "wb": wb, "bs": bs})
    return in_maps


def _prepare(latents, actions, policy_indices, W1, b1, W2, b2, W3, b3):
    latents = np.asarray(latents, dtype=np.float32)
    actions = np.asarray(actions, dtype=np.float32)
    idx = np.asarray(policy_indices).astype(np.int64)
    W1 = np.ascontiguousarray(np.asarray(W1, dtype=np.float32))
    W2 = np.ascontiguousarray(np.asarray(W2, dtype=np.float32))
    W3 = np.ascontiguousarray(np.asarray(W3, dtype=np.float32))
    b1 = np.asarray(b1, dtype=np.float32)
    b2 = np.asarray(b2, dtype=np.float32)
    b3 = np.asarray(b3, dtype=np.float32)

    order = np.argsort(idx, kind="stable")
    counts = np.bincount(idx, minlength=P).tolist()
    # pad each group to a multiple of 4 dead columns, skipped at scatter
    pcounts = [-(-n // 4) * 4 for n in counts]
    Bp = sum(pcounts)

    in_maps = _pack_inputs(
        latents, actions, order, counts, pcounts, Bp, W1, b1, W2, b2, W3, b3
    )
    nc, tiles, ntot = _build_program(pcounts, Bp, repeat=REPEAT)
    return nc, in_maps, order, counts, pcounts


def _scatter_out(results, order, counts, pcounts, B, tiles):
    Bp = sum(pcounts)
    keep = np.zeros(Bp, dtype=bool)
    po = 0
    for n, pn in zip(counts, pcounts):
        keep[po: po + n] = True
        po += pn
    out = np.empty((B, D), dtype=np.float32)
    for i in range(NCORES):
        yraw = np.asarray(results[i]["y"], dtype=np.float32)   # [128, ntot, Z]
        ysort = np.empty((Bp, Z), dtype=np.float32)
        for (g, t0, tw, s0, ns) in tiles:
            blk = yraw[:, s0: s0 + ns, :].transpose(1, 0, 2).reshape(ns * 128, Z)
            ysort[t0: t0 + tw] = blk[:tw]
        out[order, i * Z: (i + 1) * Z] = ysort[keep]
    return out


def kernel(latents, actions, policy_indices, W1, b1, W2, b2, W3, b3):
    global LAST_RESULT
    nc, in_maps, order, counts, pcounts = _prepare(
        latents, actions, policy_indices, W1, b1, W2, b2, W3, b3
    )
    tiles, _ = _group_tiles(pcounts)
    res = run_bass_kernel_spmd(nc, in_maps, list(range(NCORES)), trace=TRACE)
    LAST_RESULT = res
    B = np.asarray(latents).shape[0]
    out = _scatter_out(res.results, order, counts, pcounts, B, tiles)
    # host-side b3 add: out[b] uses policy_indices[b]'s bias for every inner i
    b3f = np.asarray(b3, dtype=np.float32)
    idx = np.asarray(policy_indices).astype(np.int64)
    out += b3f[idx].reshape(B, D)
    return out
